# revision 10
# baseline (speedup 1.0000x reference)
"""Talking-heads attention on 8 Trainium2 NeuronCores.

Sharding: data-parallel over (batch b in 0..3) x (query half in 0..1) -> 8 cores.
Each core computes K/V for its full batch sequence (1024) and attention for its
512 query rows. No collectives.

Math notes (per core, layouts transposed so contractions sit on partitions):
  - mix_pre folded into Q: qs_g[e, i] = qT[e, i] * mix_pre[h(e), g]; SCALE is
    applied as the scalar `scale` of the exp activation.
  - dotsT_g[j, i] = sum_e kT[e, j] * qs_g[e, i] over the full 768 dim.
  - softmax over j: exp on ACT; S = sum_j exp via DVE jc-tree add + gpsimd
    partition_all_reduce; R = 1/S via DVE reciprocal_approx_fast; normalize
    fused into a bf16 tensor_tensor multiply.
  - mix_post folded into V: Vt_g[j, (g',d)] = mix_post[g, g'] * v[j, (g',d)]
    with the scale row host-precomputed and pre-broadcast (vsb);
    o2[(g'd), i] += sum_j Vt_g[j, gd] * attnT_g[j, i] accumulated in PSUM over g.
  - out = o2.T @ Wout + bout.

Pipelining: PE instruction stream is [proj..., dots_0, dots_1, AV_0, dots_2,
AV_1, ..., dots_11, AV_10, AV_11, outproj] so the per-g softmax latency hides
under the previous head's AV matmuls and the PE never idles.
"""

import numpy as np

import concourse.bass as bass
import concourse.bass_isa as bass_isa
import concourse.mybir as mybir
import concourse.tile as tile
from concourse import bacc
from concourse.bass_utils import run_bass_kernel_spmd

P = 128
DIM = 768
SEQ = 1024
IQ = 512            # query rows per core
H = 12
DH = 64
NC6 = DIM // P      # 6 chunks of the 768 dim
JC8 = SEQ // P      # 8 chunks of the key dim
SCALE = DH ** -0.5
F32 = mybir.dt.float32
BF16 = mybir.dt.bfloat16

_CACHE = {}


def _build_nc():
    nc = bacc.Bacc("TRN2", target_bir_lowering=False, debug=False)

    xqT = nc.dram_tensor("xqT", [DIM, IQ], BF16, kind="ExternalInput")
    xkvT = nc.dram_tensor("xkvT", [DIM, SEQ], BF16, kind="ExternalInput")
    Wq = nc.dram_tensor("Wq", [DIM, DIM], BF16, kind="ExternalInput")
    Wk = nc.dram_tensor("Wk", [DIM, DIM], BF16, kind="ExternalInput")
    Wv = nc.dram_tensor("Wv", [DIM, DIM], BF16, kind="ExternalInput")
    Woutb = nc.dram_tensor("Woutb", [DIM, DIM], BF16, kind="ExternalInput")
    qscaleT = nc.dram_tensor("qscaleT", [P, NC6 * H], F32, kind="ExternalInput")
    vsb_in = nc.dram_tensor("vsb", [P, H * DIM], BF16, kind="ExternalInput")
    bout_t_in = nc.dram_tensor("bout_t", [P, DIM], F32, kind="ExternalInput")
    out = nc.dram_tensor("out", [IQ, DIM], F32, kind="ExternalOutput")

    r3 = lambda t: t.rearrange("(c p) e -> p c e", p=P)
    # column chunk ec of a [DIM, DIM] weight as [p, fc, 128]
    rcol = lambda t, ec: t.rearrange("(c p) e -> p c e", p=P)[:, :, ec * P:(ec + 1) * P]

    with tile.TileContext(nc) as tc:
        with (
            tc.tile_pool(name="persist", bufs=1) as pp,
        ):
            # ---- persistent tiles ----
            qT = pp.tile([P, NC6, IQ], BF16)
            kT = pp.tile([P, NC6, SEQ], BF16)
            V = pp.tile([P, JC8, DIM], BF16)     # [j-part, jc, (g,d)]
            Wout_sb = pp.tile([P, NC6, DIM], BF16)
            qscale_sb = pp.tile([P, NC6, H], F32)
            vsb = pp.tile([P, H, DIM], BF16)     # host pre-broadcast mix_post rows
            bout_sb = pp.tile([P, DIM], F32)
            o2_sb = pp.tile([P, NC6, IQ], BF16)  # o2 staged for out-proj

            # ---- phase 1: projections, DMA-overlapped ----
            with (
                tc.tile_pool(name="pin", bufs=1) as pin,
                tc.tile_pool(name="pj", bufs=2, space="PSUM") as pj,
                tc.tile_pool(name="pjv", bufs=2, space="PSUM") as pjv,
                tc.tile_pool(name="warm", bufs=1) as wp,
            ):
                xqT_sb = pin.tile([P, NC6, IQ], BF16)
                xkvT_sb = pin.tile([P, NC6, SEQ], BF16)
                Wq_sb = pin.tile([P, NC6, DIM], BF16)
                Wk_sb = pin.tile([P, NC6, DIM], BF16)
                Wv_sb = pin.tile([P, NC6, DIM], BF16)

                # PE warm-up first: zero matmuls while the first DMAs land.
                zw = wp.tile([P, 16], BF16)
                zr = wp.tile([P, IQ], BF16)
                nc.vector.memset(zw[:], 0.0)
                nc.vector.memset(zr[:], 0.0)
                wps = pj.tile([16, IQ], F32, tag="pjq", name="warmps")
                for _ in range(12):
                    nc.tensor.matmul(wps[:], zw[:], zr[:], start=True, stop=True)

                # DMA order = consumption order, single SP queue (per-DMA
                # SEQ setup is ~0.6us, so chunk only where it buys overlap).
                rcol2 = lambda t, h: t.rearrange("(c p) e -> p c e", p=P)[
                    :, :, h * (DIM // 2):(h + 1) * (DIM // 2)
                ]
                nc.sync.dma_start(xqT_sb[:], r3(xqT))
                for h2 in range(2):
                    nc.sync.dma_start(
                        Wq_sb[:, :, h2 * (DIM // 2):(h2 + 1) * (DIM // 2)],
                        rcol2(Wq, h2),
                    )
                nc.sync.dma_start(xkvT_sb[:], r3(xkvT))
                for h2 in range(2):
                    nc.sync.dma_start(
                        Wk_sb[:, :, h2 * (DIM // 2):(h2 + 1) * (DIM // 2)],
                        rcol2(Wk, h2),
                    )
                nc.sync.dma_start(Wv_sb[:], r3(Wv))
                nc.sync.dma_start(Wout_sb[:], r3(Woutb))
                nc.sync.dma_start(vsb[:], vsb_in.rearrange("p (h e) -> p h e", h=H))
                nc.sync.dma_start(
                    qscale_sb[:], qscaleT.rearrange("p (c h) -> p c h", c=NC6)
                )
                nc.sync.dma_start(bout_sb[:], bout_t_in[:, :])

                # qT[e,i] = sum_f Wq[f,e] xqT[f,i]
                for ec in range(NC6):
                    ps = pj.tile([P, IQ], F32, tag="pjq")
                    for fc in range(NC6):
                        nc.tensor.matmul(
                            ps[:], Wq_sb[:, fc, ec * P:(ec + 1) * P],
                            xqT_sb[:, fc, :], start=(fc == 0), stop=(fc == NC6 - 1),
                        )
                    nc.vector.tensor_copy(qT[:, ec, :], ps[:])

                # kT[e,j]
                for ec in range(NC6):
                    for jh in range(2):
                        ps = pj.tile([P, IQ], F32, tag="pjq")
                        for fc in range(NC6):
                            nc.tensor.matmul(
                                ps[:], Wk_sb[:, fc, ec * P:(ec + 1) * P],
                                xkvT_sb[:, fc, jh * IQ:(jh + 1) * IQ],
                                start=(fc == 0), stop=(fc == NC6 - 1),
                            )
                        nc.vector.tensor_copy(kT[:, ec, jh * IQ:(jh + 1) * IQ], ps[:])

                # V[j, gd] = sum_f xkvT[f, j] Wv[f, gd]
                for jc in range(JC8):
                    ps = pjv.tile([P, DIM], F32, tag="pjv")
                    for ns, ne in ((0, IQ), (IQ, DIM)):
                        for fc in range(NC6):
                            nc.tensor.matmul(
                                ps[:, ns:ne],
                                xkvT_sb[:, fc, jc * P:(jc + 1) * P],
                                Wv_sb[:, fc, ns:ne],
                                start=(fc == 0), stop=(fc == NC6 - 1),
                            )
                    nc.scalar.copy(V[:, jc, :], ps[:])

            # ---- phase 2: attention, g-pipelined ----
            with (
                tc.tile_pool(name="acc", bufs=1, space="PSUM") as acc,
                tc.tile_pool(name="pwork", bufs=2, space="PSUM") as pwork,
                tc.tile_pool(name="gbufs", bufs=2) as gb,
                tc.tile_pool(name="small", bufs=2) as sp,
            ):
                o2ps = [
                    acc.tile([P, IQ], F32, tag=f"o2_{s}", name=f"o2_{s}")
                    for s in range(NC6)
                ]

                def emit_qs_vt(g):
                    qs = gb.tile([P, NC6, IQ], BF16, tag="qs", name=f"qs{g}")
                    for c in range(NC6):
                        nc.vector.tensor_scalar_mul(
                            qs[:, c, :], qT[:, c, :], qscale_sb[:, c, g:g + 1]
                        )
                    vt = gb.tile([P, JC8, DIM], BF16, tag="vt", name=f"vt{g}")
                    nc.vector.tensor_tensor(
                        vt[:], V[:],
                        vsb[:, g:g + 1, :].to_broadcast((P, JC8, DIM)),
                        mybir.AluOpType.mult,
                    )
                    return qs, vt

                def emit_dots_softmax(g, qs):
                    """dots matmuls on PE; exp/S/R/norm on ACT/DVE/Pool."""
                    attnT = gb.tile([P, JC8, IQ], BF16, tag="attnT", name=f"at{g}")
                    for jb in range(JC8):
                        ds = pwork.tile([P, IQ], F32, tag="work")
                        for c in range(NC6):
                            nc.tensor.matmul(
                                ds[:], kT[:, c, jb * P:(jb + 1) * P], qs[:, c, :],
                                start=(c == 0), stop=(c == NC6 - 1),
                            )
                        nc.scalar.activation(
                            attnT[:, jb, :], ds[:],
                            mybir.ActivationFunctionType.Exp, scale=SCALE,
                        )
                    # S = sum_j attnT: jc-tree on DVE, then partition all-reduce
                    s1 = sp.tile([P, 4, IQ], BF16, tag="s1")
                    nc.vector.tensor_tensor(
                        s1[:], attnT[:, 0:4, :], attnT[:, 4:8, :],
                        mybir.AluOpType.add,
                    )
                    s2 = sp.tile([P, 2, IQ], BF16, tag="s2")
                    nc.vector.tensor_tensor(
                        s2[:], s1[:, 0:2, :], s1[:, 2:4, :], mybir.AluOpType.add
                    )
                    s3 = sp.tile([P, IQ], BF16, tag="s3")
                    nc.vector.tensor_tensor(
                        s3[:], s2[:, 0, :], s2[:, 1, :], mybir.AluOpType.add
                    )
                    Sf = sp.tile([P, IQ], F32, tag="Sf")
                    nc.gpsimd.partition_all_reduce(
                        Sf[:], s3[:], channels=P, reduce_op=bass_isa.ReduceOp.add
                    )
                    Rf = sp.tile([P, IQ], F32, tag="Rf")
                    nc.vector.reciprocal_approx_fast(out=Rf[:], in_=Sf[:])
                    R = sp.tile([P, IQ], BF16, tag="R")
                    nc.vector.tensor_copy(R[:], Rf[:])
                    nc.vector.tensor_tensor(
                        attnT[:], attnT[:],
                        R[:, None, :].to_broadcast((P, JC8, IQ)),
                        mybir.AluOpType.mult,
                    )
                    return attnT

                def emit_av(g, vt, attnT):
                    for s in range(NC6):
                        for jc in range(JC8):
                            nc.tensor.matmul(
                                o2ps[s][:],
                                vt[:, jc, s * P:(s + 1) * P],
                                attnT[:, jc, :],
                                start=(g == 0 and jc == 0),
                                stop=(g == H - 1 and jc == JC8 - 1),
                            )

                qs_cur, vt_cur = emit_qs_vt(0)
                prev = None  # (g, vt, attnT) pending AV
                for g in range(H):
                    attnT = emit_dots_softmax(g, qs_cur)
                    if g + 1 < H:
                        qs_next, vt_next = emit_qs_vt(g + 1)
                    if prev is not None:
                        emit_av(*prev)
                    prev = (g, vt_cur, attnT)
                    if g + 1 < H:
                        qs_cur, vt_cur = qs_next, vt_next
                emit_av(*prev)

                for s in range(NC6):
                    nc.scalar.copy(o2_sb[:, s, :], o2ps[s][:])

            # ---- phase 3: output projection + bias ----
            with (
                tc.tile_pool(name="pj3", bufs=2, space="PSUM") as pj3,
                tc.tile_pool(name="oBuf", bufs=2) as ob,
            ):
                CH = DIM // 3
                for isl in range(IQ // P):
                    osb = ob.tile([P, DIM], F32, tag="osb")
                    for ns in range(0, DIM, CH):
                        ne = ns + CH
                        # full-bank tile so chunks rotate PSUM banks
                        fp = pj3.tile([P, IQ], F32, tag="fin")
                        for c in range(NC6):
                            nc.tensor.matmul(
                                fp[:, :CH],
                                o2_sb[:, c, isl * P:(isl + 1) * P],
                                Wout_sb[:, c, ns:ne],
                                start=(c == 0), stop=(c == NC6 - 1),
                            )
                        nc.vector.tensor_tensor(
                            osb[:, ns:ne], fp[:, :CH], bout_sb[:, ns:ne],
                            mybir.AluOpType.add,
                        )
                        nc.sync.dma_start(
                            out[isl * P:(isl + 1) * P, ns:ne], osb[:, ns:ne]
                        )

    nc.compile()
    return nc


def _host_inputs(x, Wq, Wkv, mix_pre, mix_post, Wout, bout):
    import ml_dtypes
    bf = ml_dtypes.bfloat16

    Wk = np.ascontiguousarray(Wkv[:, :DIM]).astype(bf)
    Wv = np.ascontiguousarray(Wkv[:, DIM:]).astype(bf)

    # qscaleT[p, c*H + g] = mix_pre[h(c*128+p), g]
    heads_of_e = (np.arange(DIM) // DH)          # [768]
    s_eg = mix_pre[heads_of_e, :]                # [768, 12]
    qscaleT = np.ascontiguousarray(
        s_eg.reshape(NC6, P, H).transpose(1, 0, 2).reshape(P, NC6 * H)
    ).astype(np.float32)

    # vsb[p, g*DIM + col] = mix_post[g, col // DH]  (same for all p)
    v_row = np.repeat(mix_post, DH, axis=1).reshape(1, H * DIM)  # [1, 9216]
    vsb = np.ascontiguousarray(np.broadcast_to(v_row, (P, H * DIM))).astype(bf)

    bout_t = np.ascontiguousarray(
        np.broadcast_to(bout.reshape(1, DIM), (P, DIM))
    ).astype(np.float32)

    shared = {
        "Wq": Wq.astype(bf), "Wk": Wk, "Wv": Wv,
        "Woutb": Wout.astype(bf),
        "qscaleT": qscaleT, "vsb": vsb, "bout_t": bout_t,
    }
    in_maps = []
    for c in range(8):
        b, half = c // 2, c % 2
        m = dict(shared)
        m["xqT"] = np.ascontiguousarray(
            x[b, half * IQ:(half + 1) * IQ, :].T
        ).astype(bf)
        m["xkvT"] = np.ascontiguousarray(x[b].T).astype(bf)
        in_maps.append(m)
    return in_maps


def kernel(x, Wq, Wkv, mix_pre, mix_post, Wout, bout):
    x = np.asarray(x, dtype=np.float32)
    Wq = np.asarray(Wq, dtype=np.float32)
    Wkv = np.asarray(Wkv, dtype=np.float32)
    mix_pre = np.asarray(mix_pre, dtype=np.float32)
    mix_post = np.asarray(mix_post, dtype=np.float32)
    Wout = np.asarray(Wout, dtype=np.float32)
    bout = np.asarray(bout, dtype=np.float32)

    if "nc" not in _CACHE:
        _CACHE["nc"] = _build_nc()
    nc = _CACHE["nc"]

    in_maps = _host_inputs(x, Wq, Wkv, mix_pre, mix_post, Wout, bout)
    res = run_bass_kernel_spmd(nc, in_maps, core_ids=list(range(8)))
    _CACHE["last_results"] = res

    b_, n_, d_ = x.shape
    full = np.empty((b_, n_, d_), dtype=np.float32)
    for c in range(8):
        b, half = c // 2, c % 2
        full[b, half * IQ:(half + 1) * IQ, :] = res.results[c]["out"]
    return full


# revision 16
# speedup vs baseline: 1.0057x; 1.0057x over previous
"""Talking-heads attention on 8 Trainium2 NeuronCores.

Sharding: data-parallel over (batch b in 0..3) x (query half in 0..1) -> 8 cores.
Each core computes K/V for its full batch sequence (1024) and attention for its
512 query rows. No collectives.

Math notes (per core, layouts transposed so contractions sit on partitions):
  - mix_pre folded into Q: qs_g[e, i] = qT[e, i] * mix_pre[h(e), g]; SCALE is
    applied as the scalar `scale` of the exp activation.
  - dotsT_g[j, i] = sum_e kT[e, j] * qs_g[e, i] over the full 768 dim.
  - softmax over j: exp on ACT; S = sum_j exp via DVE jc-tree add + gpsimd
    partition_all_reduce; R = 1/S via DVE reciprocal_approx_fast; normalize
    fused into a bf16 tensor_tensor multiply.
  - mix_post folded into V: Vt_g[j, (g',d)] = mix_post[g, g'] * v[j, (g',d)]
    with the scale row host-precomputed and pre-broadcast (vsb);
    o2[(g'd), i] += sum_j Vt_g[j, gd] * attnT_g[j, i] accumulated in PSUM over g.
  - out = o2.T @ Wout + bout.

Pipelining: PE instruction stream is [proj..., dots_0, dots_1, AV_0, dots_2,
AV_1, ..., dots_11, AV_10, AV_11, outproj] so the per-g softmax latency hides
under the previous head's AV matmuls and the PE never idles.
"""

import numpy as np

import concourse.bass as bass
import concourse.bass_isa as bass_isa
import concourse.mybir as mybir
import concourse.tile as tile
from concourse import bacc
from concourse.bass_utils import run_bass_kernel_spmd

P = 128
DIM = 768
SEQ = 1024
IQ = 512            # query rows per core
H = 12
DH = 64
NC6 = DIM // P      # 6 chunks of the 768 dim
JC8 = SEQ // P      # 8 chunks of the key dim
SCALE = DH ** -0.5
F32 = mybir.dt.float32
BF16 = mybir.dt.bfloat16

_CACHE = {}


def _build_nc():
    nc = bacc.Bacc("TRN2", target_bir_lowering=False, debug=False)

    xqT = nc.dram_tensor("xqT", [DIM, IQ], BF16, kind="ExternalInput")
    xkvT = nc.dram_tensor("xkvT", [DIM, SEQ], BF16, kind="ExternalInput")
    Wq = nc.dram_tensor("Wq", [DIM, DIM], BF16, kind="ExternalInput")
    Wk = nc.dram_tensor("Wk", [DIM, DIM], BF16, kind="ExternalInput")
    Wv = nc.dram_tensor("Wv", [DIM, DIM], BF16, kind="ExternalInput")
    Woutb = nc.dram_tensor("Woutb", [DIM, DIM], BF16, kind="ExternalInput")
    qscaleT = nc.dram_tensor("qscaleT", [P, NC6 * H], F32, kind="ExternalInput")
    vsb_in = nc.dram_tensor("vsb", [P, H * DIM], BF16, kind="ExternalInput")
    bout_t_in = nc.dram_tensor("bout_t", [P, DIM], F32, kind="ExternalInput")
    out = nc.dram_tensor("out", [IQ, DIM], F32, kind="ExternalOutput")

    r3 = lambda t: t.rearrange("(c p) e -> p c e", p=P)
    # column chunk ec of a [DIM, DIM] weight as [p, fc, 128]
    rcol = lambda t, ec: t.rearrange("(c p) e -> p c e", p=P)[:, :, ec * P:(ec + 1) * P]

    with tile.TileContext(nc) as tc:
        with (
            tc.tile_pool(name="persist", bufs=1) as pp,
        ):
            # ---- persistent tiles ----
            qT = pp.tile([P, NC6, IQ], BF16)
            kT = pp.tile([P, NC6, SEQ], BF16)
            V = pp.tile([P, JC8, DIM], BF16)     # [j-part, jc, (g,d)]
            Wout_sb = pp.tile([P, NC6, DIM], BF16)
            qscale_sb = pp.tile([P, NC6, H], F32)
            vsb = pp.tile([P, H, DIM], BF16)     # host pre-broadcast mix_post rows
            bout_sb = pp.tile([P, DIM], F32)
            o2_sb = pp.tile([P, NC6, IQ], BF16)  # o2 staged for out-proj

            # ---- phase 1: projections, DMA-overlapped ----
            # k-proj runs first in fc-waves chasing its own (xkvT, Wk) chunk
            # DMAs; q/v weight DMAs land underneath the k/q matmuls.
            with (
                tc.tile_pool(name="pin", bufs=1) as pin,
                tc.tile_pool(name="warm", bufs=1) as wp,
            ):
                xqT_sb = pin.tile([P, NC6, IQ], BF16)
                xkvT_sb = pin.tile([P, NC6, SEQ], BF16)
                Wq_sb = pin.tile([P, NC6, DIM], BF16)
                Wk_sb = pin.tile([P, NC6, DIM], BF16)
                Wv_sb = pin.tile([P, NC6, DIM], BF16)

                # PE warm-up: zero matmuls while the first DMA chunks land.
                zw = wp.tile([P, 16], BF16)
                zr = wp.tile([P, IQ], BF16)
                nc.vector.memset(zw[:], 0.0)
                nc.vector.memset(zr[:], 0.0)

                # row-chunk fc of a [DIM, N] dram tensor -> [p, 1, N]
                rrow = lambda t, fc: t.rearrange("(c p) e -> p c e", p=P)[
                    :, fc:fc + 1, :
                ]
                rcol2 = lambda t, h: t.rearrange("(c p) e -> p c e", p=P)[
                    :, :, h * (DIM // 2):(h + 1) * (DIM // 2)
                ]
                # DMA order = consumption order, single SP queue (per-DMA
                # SEQ setup is ~0.6us, so chunk only where it buys overlap).
                for fc in range(NC6):
                    nc.sync.dma_start(xkvT_sb[:, fc:fc + 1, :], rrow(xkvT, fc))
                    nc.sync.dma_start(Wk_sb[:, fc:fc + 1, :], rrow(Wk, fc))
                nc.sync.dma_start(xqT_sb[:], r3(xqT))
                for h2 in range(2):
                    nc.sync.dma_start(
                        Wq_sb[:, :, h2 * (DIM // 2):(h2 + 1) * (DIM // 2)],
                        rcol2(Wq, h2),
                    )
                nc.sync.dma_start(Wv_sb[:], r3(Wv))
                nc.sync.dma_start(Wout_sb[:], r3(Woutb))
                nc.sync.dma_start(vsb[:], vsb_in.rearrange("p (h e) -> p h e", h=H))
                nc.sync.dma_start(
                    qscale_sb[:], qscaleT.rearrange("p (c h) -> p c h", c=NC6)
                )
                nc.sync.dma_start(bout_sb[:], bout_t_in[:, :])

                with tc.tile_pool(name="pj", bufs=1, space="PSUM") as pj:
                    # warm-up matmuls share the kq0 bank
                    wps = pj.tile([P, IQ], F32, tag="kq0", name="warmps")
                    for _ in range(6):
                        nc.tensor.matmul(
                            wps[0:16, :], zw[:], zr[:], start=True, stop=True
                        )

                    # kT[e,j]: fc-outer waves over 6 parallel ec accumulators,
                    # two jh passes reusing the resident chunks.
                    for jh in range(2):
                        kps = [
                            pj.tile([P, IQ], F32, tag=f"kq{ec}", name=f"kps{ec}")
                            for ec in range(NC6)
                        ]
                        for fc in range(NC6):
                            for ec in range(NC6):
                                nc.tensor.matmul(
                                    kps[ec][:], Wk_sb[:, fc, ec * P:(ec + 1) * P],
                                    xkvT_sb[:, fc, jh * IQ:(jh + 1) * IQ],
                                    start=(fc == 0), stop=(fc == NC6 - 1),
                                )
                        for ec in range(NC6):
                            nc.vector.tensor_copy(
                                kT[:, ec, jh * IQ:(jh + 1) * IQ], kps[ec][:]
                            )

                    # qT[e,i] = sum_f Wq[f,e] xqT[f,i] (weights resident)
                    for ec in range(NC6):
                        ps = pj.tile([P, IQ], F32, tag=f"kq{ec}")
                        for fc in range(NC6):
                            nc.tensor.matmul(
                                ps[:], Wq_sb[:, fc, ec * P:(ec + 1) * P],
                                xqT_sb[:, fc, :],
                                start=(fc == 0), stop=(fc == NC6 - 1),
                            )
                        nc.vector.tensor_copy(qT[:, ec, :], ps[:])

                # V[j, gd] = sum_f xkvT[f, j] Wv[f, gd]
                with tc.tile_pool(name="pjv", bufs=2, space="PSUM") as pjv:
                    for jc in range(JC8):
                        ps = pjv.tile([P, DIM], F32, tag="pjv")
                        for ns, ne in ((0, IQ), (IQ, DIM)):
                            for fc in range(NC6):
                                nc.tensor.matmul(
                                    ps[:, ns:ne],
                                    xkvT_sb[:, fc, jc * P:(jc + 1) * P],
                                    Wv_sb[:, fc, ns:ne],
                                    start=(fc == 0), stop=(fc == NC6 - 1),
                                )
                        nc.scalar.copy(V[:, jc, :], ps[:])

            # ---- phase 2: attention, g-pipelined ----
            with (
                tc.tile_pool(name="acc", bufs=1, space="PSUM") as acc,
                tc.tile_pool(name="pwork", bufs=2, space="PSUM") as pwork,
                tc.tile_pool(name="gbufs", bufs=2) as gb,
                tc.tile_pool(name="small", bufs=2) as sp,
            ):
                o2ps = [
                    acc.tile([P, IQ], F32, tag=f"o2_{s}", name=f"o2_{s}")
                    for s in range(NC6)
                ]

                def emit_qs_vt(g):
                    qs = gb.tile([P, NC6, IQ], BF16, tag="qs", name=f"qs{g}")
                    for c in range(NC6):
                        nc.vector.tensor_scalar_mul(
                            qs[:, c, :], qT[:, c, :], qscale_sb[:, c, g:g + 1]
                        )
                    vt = gb.tile([P, JC8, DIM], BF16, tag="vt", name=f"vt{g}")
                    nc.vector.tensor_tensor(
                        vt[:], V[:],
                        vsb[:, g:g + 1, :].to_broadcast((P, JC8, DIM)),
                        mybir.AluOpType.mult,
                    )
                    return qs, vt

                def emit_dots_softmax(g, qs):
                    """dots matmuls on PE; exp/S/R/norm on ACT/DVE/Pool."""
                    attnT = gb.tile([P, JC8, IQ], BF16, tag="attnT", name=f"at{g}")
                    for jb in range(JC8):
                        ds = pwork.tile([P, IQ], F32, tag="work")
                        for c in range(NC6):
                            nc.tensor.matmul(
                                ds[:], kT[:, c, jb * P:(jb + 1) * P], qs[:, c, :],
                                start=(c == 0), stop=(c == NC6 - 1),
                            )
                        nc.scalar.activation(
                            attnT[:, jb, :], ds[:],
                            mybir.ActivationFunctionType.Exp, scale=SCALE,
                        )
                    # S = sum_j attnT: jc-tree on DVE, then partition all-reduce
                    s1 = sp.tile([P, 4, IQ], BF16, tag="s1")
                    nc.vector.tensor_tensor(
                        s1[:], attnT[:, 0:4, :], attnT[:, 4:8, :],
                        mybir.AluOpType.add,
                    )
                    s2 = sp.tile([P, 2, IQ], BF16, tag="s2")
                    nc.vector.tensor_tensor(
                        s2[:], s1[:, 0:2, :], s1[:, 2:4, :], mybir.AluOpType.add
                    )
                    s3 = sp.tile([P, IQ], BF16, tag="s3")
                    nc.vector.tensor_tensor(
                        s3[:], s2[:, 0, :], s2[:, 1, :], mybir.AluOpType.add
                    )
                    Sf = sp.tile([P, IQ], F32, tag="Sf")
                    nc.gpsimd.partition_all_reduce(
                        Sf[:], s3[:], channels=P, reduce_op=bass_isa.ReduceOp.add
                    )
                    Rf = sp.tile([P, IQ], F32, tag="Rf")
                    nc.vector.reciprocal_approx_fast(out=Rf[:], in_=Sf[:])
                    R = sp.tile([P, IQ], BF16, tag="R")
                    nc.vector.tensor_copy(R[:], Rf[:])
                    nc.vector.tensor_tensor(
                        attnT[:], attnT[:],
                        R[:, None, :].to_broadcast((P, JC8, IQ)),
                        mybir.AluOpType.mult,
                    )
                    return attnT

                def emit_av(g, vt, attnT):
                    for s in range(NC6):
                        for jc in range(JC8):
                            nc.tensor.matmul(
                                o2ps[s][:],
                                vt[:, jc, s * P:(s + 1) * P],
                                attnT[:, jc, :],
                                start=(g == 0 and jc == 0),
                                stop=(g == H - 1 and jc == JC8 - 1),
                            )

                qs_cur, vt_cur = emit_qs_vt(0)
                prev = None  # (g, vt, attnT) pending AV
                for g in range(H):
                    attnT = emit_dots_softmax(g, qs_cur)
                    if g + 1 < H:
                        qs_next, vt_next = emit_qs_vt(g + 1)
                    if prev is not None:
                        emit_av(*prev)
                    prev = (g, vt_cur, attnT)
                    if g + 1 < H:
                        qs_cur, vt_cur = qs_next, vt_next
                emit_av(*prev)

                for s in range(NC6):
                    nc.scalar.copy(o2_sb[:, s, :], o2ps[s][:])

            # ---- phase 3: output projection + bias ----
            with (
                tc.tile_pool(name="pj3", bufs=2, space="PSUM") as pj3,
                tc.tile_pool(name="oBuf", bufs=2) as ob,
            ):
                CH = DIM // 3
                for isl in range(IQ // P):
                    osb = ob.tile([P, DIM], F32, tag="osb")
                    for ns in range(0, DIM, CH):
                        ne = ns + CH
                        # full-bank tile so chunks rotate PSUM banks
                        fp = pj3.tile([P, IQ], F32, tag="fin")
                        for c in range(NC6):
                            nc.tensor.matmul(
                                fp[:, :CH],
                                o2_sb[:, c, isl * P:(isl + 1) * P],
                                Wout_sb[:, c, ns:ne],
                                start=(c == 0), stop=(c == NC6 - 1),
                            )
                        nc.vector.tensor_tensor(
                            osb[:, ns:ne], fp[:, :CH], bout_sb[:, ns:ne],
                            mybir.AluOpType.add,
                        )
                        nc.sync.dma_start(
                            out[isl * P:(isl + 1) * P, ns:ne], osb[:, ns:ne]
                        )

    nc.compile()
    return nc


def _host_inputs(x, Wq, Wkv, mix_pre, mix_post, Wout, bout):
    import ml_dtypes
    bf = ml_dtypes.bfloat16

    Wk = np.ascontiguousarray(Wkv[:, :DIM]).astype(bf)
    Wv = np.ascontiguousarray(Wkv[:, DIM:]).astype(bf)

    # qscaleT[p, c*H + g] = mix_pre[h(c*128+p), g]
    heads_of_e = (np.arange(DIM) // DH)          # [768]
    s_eg = mix_pre[heads_of_e, :]                # [768, 12]
    qscaleT = np.ascontiguousarray(
        s_eg.reshape(NC6, P, H).transpose(1, 0, 2).reshape(P, NC6 * H)
    ).astype(np.float32)

    # vsb[p, g*DIM + col] = mix_post[g, col // DH]  (same for all p)
    v_row = np.repeat(mix_post, DH, axis=1).reshape(1, H * DIM)  # [1, 9216]
    vsb = np.ascontiguousarray(np.broadcast_to(v_row, (P, H * DIM))).astype(bf)

    bout_t = np.ascontiguousarray(
        np.broadcast_to(bout.reshape(1, DIM), (P, DIM))
    ).astype(np.float32)

    shared = {
        "Wq": Wq.astype(bf), "Wk": Wk, "Wv": Wv,
        "Woutb": Wout.astype(bf),
        "qscaleT": qscaleT, "vsb": vsb, "bout_t": bout_t,
    }
    in_maps = []
    for c in range(8):
        b, half = c // 2, c % 2
        m = dict(shared)
        m["xqT"] = np.ascontiguousarray(
            x[b, half * IQ:(half + 1) * IQ, :].T
        ).astype(bf)
        m["xkvT"] = np.ascontiguousarray(x[b].T).astype(bf)
        in_maps.append(m)
    return in_maps


def kernel(x, Wq, Wkv, mix_pre, mix_post, Wout, bout):
    x = np.asarray(x, dtype=np.float32)
    Wq = np.asarray(Wq, dtype=np.float32)
    Wkv = np.asarray(Wkv, dtype=np.float32)
    mix_pre = np.asarray(mix_pre, dtype=np.float32)
    mix_post = np.asarray(mix_post, dtype=np.float32)
    Wout = np.asarray(Wout, dtype=np.float32)
    bout = np.asarray(bout, dtype=np.float32)

    if "nc" not in _CACHE:
        _CACHE["nc"] = _build_nc()
    nc = _CACHE["nc"]

    in_maps = _host_inputs(x, Wq, Wkv, mix_pre, mix_post, Wout, bout)
    try:
        res = run_bass_kernel_spmd(nc, in_maps, core_ids=list(range(8)))
    except Exception:
        # transient device wedges (NRT_EXEC_UNIT_UNRECOVERABLE) recover on
        # a clean retry
        res = run_bass_kernel_spmd(nc, in_maps, core_ids=list(range(8)))
    _CACHE["last_results"] = res

    b_, n_, d_ = x.shape
    full = np.empty((b_, n_, d_), dtype=np.float32)
    for c in range(8):
        b, half = c // 2, c % 2
        full[b, half * IQ:(half + 1) * IQ, :] = res.results[c]["out"]
    return full
